# revision 43
# baseline (speedup 1.0000x reference)
"""Causal multi-head attention with RoPE for TRN2 (Bass/Tile), 8 NeuronCores.

Problem: y = (softmax(causal(rope(x@Wq) @ rope(x@Wk)^T / sqrt(dh))) @ (x@Wv)) @ Wo
  B=4, T=2048, D=2048, H=16 heads, dh=128, fp32 I/O.

Sharding: 4-way batch data-parallel x 2-way head tensor-parallel.
  Core c: batch b = c//2, head group g = c%2 (heads 8g..8g+7).
  Each core computes a partial y[b] (its 8 heads' contribution through Wo);
  the host sums the two partials per batch.

Head-pipelined schedule: the Act-engine exp stream of head h's attention is
hidden under the PE-bound Q/K projection of head h+1 (and under the Wo output
projection for the last head) by interleaving instruction emission. Q/K never
round-trip through DRAM: per-head [128, t] tiles live in SBUF with 2-deep
head rotation. V is projected in a prologue (PE-bound, nothing to hide yet).

All matmuls run in fp16 (full PE rate; fp32 PSUM accumulation), transposed
layouts throughout (no on-chip transposes):
  - Projections contract over D with x^T resident in SBUF: Q^T/K^T produced as
    [dh, t]; V as [t, dh].
  - S^T[k, q] = (K^T chunk) stationary against Q^T moving; exp(S^T) is
    directly the moving operand of the P@V matmul -> O^T [dh, q], which is
    directly the moving operand of the Wo projection.
Causal diagonal 128-blocks are computed at partial width (S, exp, PV and the
rowsum all skip fully-masked columns); only the triangular first 128 valid
columns of each diagonal chunk need a mask multiply.
Softmax: no max subtraction (logits are O(+-6), exp is fp32-safe); the
denominator comes from a ones-vector matmul over DVE pair-tree partial sums;
each quad's ones-matmul is deferred one block so PE never waits on the tree.
RoPE: pair partners are pre-permuted into partition halves (even dh dims ->
partitions 0..63, odd -> 64..127) via a host-side column permutation of Wq/Wk,
making rotate-half a uniform +-64-partition shift on chip.
"""

import numpy as np

import concourse.bass as bass
import concourse.tile as tile
from concourse import bacc, mybir
from concourse.bass import ts
from concourse.bass_utils import run_bass_kernel_spmd

B, T, D = 4, 2048, 2048
H = 16
DH = 128
THETA = 10000.0
NCORES = 8
HPC = H // 2  # heads per core (2-way head TP)
P = 128
TQ = 512  # q-tile width
F16 = mybir.dt.float16
F32 = mybir.dt.float32


def build(t=T, d=D, hpc=HPC, reps=1, mmdt=None, trunc=None, mut=None):
    """Build the per-core Bass program (same program on all cores).

    reps>1 wraps the whole computation in a hardware loop (timing builds).
    trunc: None=full, "P"=prologue only, k (int)=prologue + k slots,
    "A"=everything except the outproj blocks (timing bisection builds).
    mut: timing-experiment mutations (break numerics): "noexp" (DVE copy
    instead of Act exp), "noil" (no proj/attn interleave), "noattn"/"noproj"
    (slots emit only one side), "norow" (skip rowsum tree+ones+recip).
    """
    nc = bacc.Bacc("TRN2", target_bir_lowering=False, debug=False)
    MMDT = mmdt or F16
    dc = d // P  # contraction chunks for projections
    tc_n = t // P  # token chunks (k-chunks in attention)
    ntq = t // TQ  # q tiles
    nnt = d // P  # output-projection row chunks
    vg = 2  # V head-groups (4 heads each, N=512)
    vgh = hpc // vg
    vgw = vgh * DH

    xt = nc.dram_tensor("xt", [d, t], MMDT, kind="ExternalInput").ap()
    wq = nc.dram_tensor("wq", [hpc, d, DH], MMDT, kind="ExternalInput").ap()
    wk = nc.dram_tensor("wk", [hpc, d, DH], MMDT, kind="ExternalInput").ap()
    wv = nc.dram_tensor("wv", [vg, d, vgw], MMDT, kind="ExternalInput").ap()
    wo = nc.dram_tensor("wo", [hpc * DH, d], MMDT, kind="ExternalInput").ap()
    cos = nc.dram_tensor("cos", [P, t], F16, kind="ExternalInput").ap()
    sin = nc.dram_tensor("sin", [P, t], F16, kind="ExternalInput").ap()
    tri = nc.dram_tensor("tri", [P, P], MMDT, kind="ExternalInput").ap()
    ones = nc.dram_tensor("ones", [P, P], MMDT, kind="ExternalInput").ap()
    y = nc.dram_tensor("y", [d, t], F16, kind="ExternalOutput").ap()
    wo_r = wo.rearrange("(h p) n -> p h n", p=P)

    with tile.TileContext(nc) as tc:
        import contextlib

        loop_cm = tc.For_i(0, reps, 1) if reps > 1 else contextlib.nullcontext()
        with (
            loop_cm,
            tc.tile_pool(name="vpool", bufs=1) as vp,
            tc.tile_pool(name="const", bufs=1) as constp,
            tc.tile_pool(name="xt", bufs=1) as xtp,
            tc.tile_pool(name="qk", bufs=2) as qkp,
            tc.tile_pool(name="wstream", bufs=2) as wsp,
            tc.tile_pool(name="rope", bufs=1) as rp_,
            tc.tile_pool(name="exps", bufs=2) as esp,
            tc.tile_pool(name="tree", bufs=2) as trp,
            tc.tile_pool(name="small", bufs=2) as smp,
            tc.tile_pool(name="qkpsum", bufs=1, space="PSUM") as qkpsp,
            tc.tile_pool(name="spsum", bufs=4, space="PSUM") as spsp,
            tc.tile_pool(name="opsum", bufs=1, space="PSUM") as opsp,
            tc.tile_pool(name="rpsum", bufs=1, space="PSUM") as rpsp,
        ):
            v_sb = vp.tile([P, tc_n, vg * vgw], MMDT, tag="v", name="v")
            cos_sb = constp.tile([P, t], F16, tag="cos")
            sin_sb = constp.tile([P, t], F16, tag="sin")
            tri_sb = constp.tile([P, P], MMDT, tag="tri")
            ones_sb = constp.tile([P, P], MMDT, tag="ones")
            xt_sb = xtp.tile([P, dc, t], MMDT, tag="xt")

            def load_consts():
                nc.sync.dma_start(cos_sb[:], cos)
                nc.sync.dma_start(sin_sb[:], sin)
                nc.sync.dma_start(tri_sb[:], tri)
                nc.sync.dma_start(ones_sb[:], ones)

            def load_x(q4s):
                for q4 in q4s:
                    for c in range(dc):
                        nc.sync.dma_start(
                            xt_sb[:, c, ts(q4, TQ)], xt[ts(c, P), ts(q4, TQ)]
                        )

            def load_w(h):
                out = []
                for name, w_ap in (("q", wq[h]), ("k", wk[h])):
                    w_sb = wsp.tile([P, dc, DH], MMDT, tag=f"w{name}")
                    nc.sync.dma_start(
                        w_sb[:], w_ap.rearrange("(c p) m -> p c m", p=P)
                    )
                    out.append(w_sb)
                return out

            def rope_pair_drain(pq, jt, qk_sb):
                # rope on the paired q||k psum tile [P, 2, TQ] in 4 DVE ops:
                # out = pq*cos + swap(pq)*nsin, nsin's top half pre-negated
                # host-side so rotate-half is two plain muls with crossed
                # partition halves (scalar_tensor_tensor is pathologically
                # slow on hw and avoided; op COUNT matters - each DVE op
                # carries ~300ns fixed overhead).
                def bc(tab, p0, p1):
                    return tab[p0:p1, ts(jt, TQ)].unsqueeze(1).broadcast_to(
                        [p1 - p0, 2, TQ]
                    )

                rot = rp_.tile([P, 2, TQ], F16, tag="rot")
                nc.vector.tensor_mul(rot[0:64], pq[64:128], bc(sin_sb, 0, 64))
                nc.vector.tensor_mul(rot[64:128], pq[0:64], bc(sin_sb, 64, 128))
                t1 = rp_.tile([P, 2, TQ], F16, tag="t1")
                nc.vector.tensor_mul(t1[:], pq[:], bc(cos_sb, 0, 128))
                nc.vector.tensor_add(qk_sb[:, jt], t1[:], rot[:])

            def proj_qk_blocks(w_pair, qk_sb):
                """8 closures per head: per jt a q-chain piece and a k-chain +
                rope piece, projecting into the paired psum tile."""
                blocks = []
                for jt in range(ntq):
                    pq_box = [None]

                    def blk_q(jt=jt, pq_box=pq_box):
                        pq_box[0] = qkpsp.tile([P, 2, TQ], F32, tag="pq", name="pq")
                        for c in range(dc):
                            nc.tensor.matmul(
                                pq_box[0][:, 0],
                                w_pair[0][:, c, :],
                                xt_sb[:, c, ts(jt, TQ)],
                                start=(c == 0),
                                stop=(c == dc - 1),
                            )

                    def blk_k(jt=jt, pq_box=pq_box):
                        pq = pq_box[0]
                        for c in range(dc):
                            nc.tensor.matmul(
                                pq[:, 1],
                                w_pair[1][:, c, :],
                                xt_sb[:, c, ts(jt, TQ)],
                                start=(c == 0),
                                stop=(c == dc - 1),
                            )
                        rope_pair_drain(pq, jt, qk_sb)

                    blocks.append(blk_q)
                    blocks.append(blk_k)
                return blocks

            def load_wv(wvp):
                # chunk-granular DMAs interleaved with x's first q-tile so the
                # first V-projection matmul can start after ~2 chunks arrive
                wv_all = [
                    wvp.tile([P, dc, vgw], MMDT, tag=f"wv{g}", name=f"wv{g}")
                    for g in range(vg)
                ]
                wv_r = [wv[g].rearrange("(c p) m -> p c m", p=P) for g in range(vg)]
                for c in range(dc):
                    nc.sync.dma_start(
                        xt_sb[:, c, ts(0, TQ)], xt[ts(c, P), ts(0, TQ)]
                    )
                    for g in range(vg):
                        nc.sync.dma_start(wv_all[g][:, c, :], wv_r[g][:, c, :])
                return wv_all

            def proj_v(wv_all):
                # uses the (idle in prologue) sp psum pool: 4 rotating banks
                # so each group's drain overlaps later chains
                for tt in range(tc_n):
                    pvs = [spsp.tile([P, vgw], F32, tag="sp", name=f"pv{g}")
                           for g in range(vg)]
                    for g in range(vg):
                        for c in range(dc):
                            nc.tensor.matmul(
                                pvs[g][:],
                                xt_sb[:, c, ts(tt, P)],
                                wv_all[g][:, c, :],
                                start=(c == 0),
                                stop=(c == dc - 1),
                            )
                        nc.vector.tensor_copy(
                            v_sb[:, tt, g * vgw : (g + 1) * vgw], pvs[g][:]
                        )

            # ---------------- attention for one head ----------------
            def attn_blocks(h, qk_sb, aot_sb):
                """Returns [(kind, closure)] with kind in {"s","pv","end"};
                the merger places a proj/outproj piece between every s-burst
                and its pv-burst so the Act exp latency is always covered by
                PE work."""
                g, hh = divmod(h, vgh)

                def kslice(c):
                    return qk_sb[:, c // 4, 1, (c % 4) * P : (c % 4 + 1) * P]

                blocks = []
                for jt in range(ntq):
                    nch = (jt + 1) * (TQ // P)
                    nquad = nch // 4
                    state = {"sps": {}, "esq": None, "pend": [],
                             "op": None, "rp": None}

                    def s_burst(q, jt=jt, state=state):
                        if q == 0:
                            state["op"] = opsp.tile([P, TQ], F32, tag="op", name="op")
                            state["rp"] = rpsp.tile([P, TQ], F32, tag="rp", name="rp")
                        for c in range(4 * q, 4 * q + 4):
                            o = c - jt * (TQ // P)
                            w0 = max(0, o) * P  # first valid column
                            sp = spsp.tile([P, TQ], F32, tag="sp", name="sp")
                            nc.tensor.matmul(
                                sp[:, w0:TQ],
                                kslice(c),
                                qk_sb[:, jt, 0, w0:TQ],
                                start=True,
                                stop=True,
                            )
                            state["sps"][c] = (sp, w0)

                    def pv_burst(q, nquad, jt=jt, state=state, nch=nch):
                        # exp + mask + PV for chunks of quad q into a single
                        # [P, 4, TQ] quad tile (fewer, larger DVE tree ops);
                        # quad q's ones-matmul is deferred into the next block
                        # so PE never waits on the tree.
                        cs = list(range(4 * q, 4 * q + 4))
                        esq = esp.tile([P, 4, TQ], MMDT, tag="esq")
                        diag = cs[0] - jt * (TQ // P) >= 0
                        for i, c in enumerate(cs):
                            sp, w0 = state["sps"].pop(c)
                            if mut == "noexp":
                                nc.vector.tensor_copy(esq[:, i, w0:TQ], sp[:, w0:TQ])
                            else:
                                nc.scalar.activation(
                                    esq[:, i, w0:TQ], sp[:, w0:TQ],
                                    mybir.ActivationFunctionType.Exp,
                                )
                            if diag:
                                # triangular mask on the first valid 128 cols
                                nc.vector.tensor_mul(
                                    esq[:, i, w0 : w0 + P],
                                    esq[:, i, w0 : w0 + P],
                                    tri_sb[:],
                                )
                            nc.tensor.matmul(
                                state["op"][:, w0:TQ],
                                v_sb[:, c, g * vgw + hh * DH : g * vgw + (hh + 1) * DH],
                                esq[:, i, w0:TQ],
                                start=(c == 0),
                                stop=(c == nch - 1),
                            )
                        if mut == "norow":
                            return
                        # rowsum tree (off PE critical path)
                        if diag:
                            # cascade partial widths into slice 0
                            for i in range(1, 4):
                                w0 = i * P
                                nc.vector.tensor_add(
                                    esq[:, 0, w0:TQ], esq[:, 0, w0:TQ],
                                    esq[:, i, w0:TQ],
                                )
                            equad = esq[:, 0, :]
                        else:
                            th = trp.tile([P, 2, TQ], MMDT, tag="th")
                            nc.vector.tensor_add(
                                th[:], esq[:, 0:2, :], esq[:, 2:4, :]
                            )
                            eq = trp.tile([P, TQ], MMDT, tag="eq")
                            nc.vector.tensor_add(eq[:], th[:, 0, :], th[:, 1, :])
                            equad = eq[:]

                        if state["pend"]:
                            state["pend"].pop(0)()

                        def ones_mm(q=q, equad=equad, nquad=nquad, state=state):
                            nc.tensor.matmul(
                                state["rp"][:],
                                ones_sb[:],
                                equad,
                                start=(q == 0),
                                stop=(q == nquad - 1),
                            )
                        state["pend"].append(ones_mm)

                    def jt_end(h=h, jt=jt, state=state, aot_sb=aot_sb):
                        if mut == "norow":
                            nc.vector.tensor_copy(
                                aot_sb[:, h, ts(jt, TQ)], state["op"][:]
                            )
                            return
                        while state["pend"]:
                            state["pend"].pop(0)()
                        rs = smp.tile([P, TQ], F32, tag="rs")
                        nc.vector.reciprocal(rs[:], state["rp"][:])
                        nc.vector.tensor_mul(
                            aot_sb[:, h, ts(jt, TQ)], state["op"][:], rs[:]
                        )

                    for q in range(nquad):
                        blocks.append(("s", lambda f=s_burst, q=q: f(q)))
                        blocks.append(("pv", lambda f=pv_burst, q=q, nq=nquad: f(q, nq)))
                    blocks.append(("end", jt_end))
                return blocks

            y_r = y.rearrange("(c p) t -> p c t", p=P)

            def outproj_blocks(jt, aot_sb, wop, cdp):
                """Output projection columns tq=jt: nnt/2 block closures, one
                nt-PAIR each (two 8-head accumulation chains into the paired
                pq psum tile - idle in the final slot - one paired drain +
                DMA); wo streamed by nt with prefetch (re-streamed each jt:
                4 MB x 4, fully hidden)."""
                wo_tiles = {}

                def load(nt):
                    wo_nt = wop.tile([P, hpc, P], MMDT, tag="wo")
                    nc.sync.dma_start(wo_nt[:], wo_r[:, :, ts(nt, P)])
                    wo_tiles[nt] = wo_nt

                def blk(nt, jt=jt):
                    if nt == 0:
                        for i in range(6):
                            load(i)
                    yp = qkpsp.tile([P, 2, TQ], F32, tag="pq", name="yp")
                    for a in range(2):
                        wo_nt = wo_tiles.pop(nt + a)
                        for h in range(hpc):
                            nc.tensor.matmul(
                                yp[:, a],
                                wo_nt[:, h, :],
                                aot_sb[:, h, ts(jt, TQ)],
                                start=(h == 0),
                                stop=(h == hpc - 1),
                            )
                        if nt + a + 6 < nnt:
                            load(nt + a + 6)
                    ytile = cdp.tile([P, 2, TQ], F16, tag="ytile")
                    nc.vector.tensor_copy(ytile[:], yp[:])
                    nc.sync.dma_start(
                        y_r[:, nt : nt + 2, ts(jt, TQ)], ytile[:]
                    )

                return [lambda nt=nt: blk(nt) for nt in range(0, nnt, 2)]

            def interleave(primary, secondary):
                """Emit tagged primary (attn) blocks with secondary (proj /
                outproj) pieces spliced so that one piece lands between every
                s-burst and its pv-burst (covering the Act exp latency with
                PE work), surplus drained proportionally at jt ends."""
                ns_ = len(secondary)
                npv = sum(1 for k, _ in primary if k == "pv") or 1
                si = 0
                pvi = 0
                for kind, blk in primary:
                    if kind == "pv":
                        pvi += 1
                        want = min(ns_, -(-pvi * ns_ // npv))  # ceil
                        while si < want:
                            secondary[si]()
                            si += 1
                    blk()
                while si < ns_:
                    secondary[si]()
                    si += 1

            # ======================= schedule =======================
            # prologue: x/V (PE-bound, nothing to hide) + head-0 Q/K
            wvp_cm = tc.tile_pool(name="wvp", bufs=1)
            wvp = wvp_cm.__enter__()
            wv_all = load_wv(wvp)
            load_consts()
            load_x([1, 2, 3])
            w_cur = load_w(0)
            proj_v(wv_all)
            cur_qk = qkp.tile([P, ntq, 2, TQ], MMDT, tag="qk")
            for blk in proj_qk_blocks(w_cur, cur_qk):
                blk()
            wvp_cm.__exit__(None, None, None)

            # aot + slot-7 pools open after wv's SBUF is released
            aot_cm = tc.tile_pool(name="aot", bufs=1)
            aotp = aot_cm.__enter__()
            wop_cm = tc.tile_pool(name="wostream", bufs=6)
            wop = wop_cm.__enter__()
            cdp_cm = tc.tile_pool(name="cdrain", bufs=2)
            cdp = cdp_cm.__enter__()
            aot_sb = aotp.tile([P, hpc, t], MMDT, tag="aot")

            # slots 1..7: attn(h-1) interleaved with proj_qk(h)
            nslot = hpc if trunc in (None, "A") else (1 if trunc == "P" else trunc + 1)
            w_next = load_w(1)
            for h in range(1, nslot):
                w_cur = w_next
                nxt_qk = qkp.tile([P, ntq, 2, TQ], MMDT, tag="qk")
                pb = proj_qk_blocks(w_cur, nxt_qk)
                if h + 1 < hpc:
                    w_next = load_w(h + 1)
                ab = attn_blocks(h - 1, cur_qk, aot_sb)
                if mut == "noattn":
                    ab = []
                elif mut == "noproj":
                    pb = []
                if mut == "noil":
                    for _, blk in ab:
                        blk()
                    for blk in pb:
                        blk()
                else:
                    interleave(ab, pb)
                cur_qk = nxt_qk

            # final slot: attn(7), with outproj(jt-1) interleaved into the
            # attn jt group so the jt_end -> outproj dependency wait is hidden
            if trunc in (None, "A"):
                ab = attn_blocks(hpc - 1, cur_qk, aot_sb)
                groups = []
                abi = 0
                for jt in range(ntq):
                    n = 2 * (jt + 1) + 1
                    groups.append(ab[abi : abi + n])
                    abi += n
                assert abi == len(ab)
                for _, blk in groups[0]:
                    blk()
                for jt in range(1, ntq):
                    interleave(
                        groups[jt],
                        outproj_blocks(jt - 1, aot_sb, wop, cdp)
                        if trunc is None else [],
                    )
                if trunc is None:
                    for blk in outproj_blocks(ntq - 1, aot_sb, wop, cdp):
                        blk()

            cdp_cm.__exit__(None, None, None)
            wop_cm.__exit__(None, None, None)
            aot_cm.__exit__(None, None, None)

    nc.compile()
    return nc


def _rope_tables(t=T):
    """cos/sin in transposed+permuted layout [128, t] (fp16).

    Partition p < 64 holds dh dim 2p (even), p >= 64 holds dh dim 2(p-64)+1;
    pair (2i, 2i+1) shares inv_freq[i], so row p uses inv_freq[p % 64].
    """
    inv_freq = 1.0 / (THETA ** (np.arange(0, DH, 2, dtype=np.float64) / DH))  # [64]
    pos = np.arange(t, dtype=np.float64)
    freqs = pos[None, :] * inv_freq[np.arange(P) % 64][:, None]  # [128, t]
    sin = np.sin(freqs)
    sin[:64] = -sin[:64]  # rotate-half sign baked into the table's top half
    return (
        np.cos(freqs).astype(np.float16),
        sin.astype(np.float16),
    )


def _perm():
    """Within-head dh permutation: even dims first, then odd dims."""
    return np.concatenate([np.arange(0, DH, 2), np.arange(1, DH, 2)])


def _tri():
    """tri[dk, dq] = 1 if dk <= dq else 0 (within-chunk causal triangle)."""
    dk = np.arange(P)[:, None]
    dq = np.arange(P)[None, :]
    return (dk <= dq).astype(np.float16)


def prep_core_inputs(x_b, Wq_g, Wk_g, Wv_g, Wo_g, t=T, hpc=HPC, npdt=np.float16):
    """Host-side input prep for one core.

    x_b: [t, D] (this core's batch); W*_g: this core's head-group slices
    (Wq/Wk/Wv: [D, hpc*DH] columns, Wo: [hpc*DH, D] rows).
    """
    d = x_b.shape[1]
    perm = _perm()
    scale = 1.0 / np.sqrt(DH)
    vg = 2
    vgw = (hpc // vg) * DH

    wq = np.empty((hpc, d, DH), npdt)
    wk = np.empty((hpc, d, DH), npdt)
    for h in range(hpc):
        blk_q = Wq_g[:, h * DH : (h + 1) * DH]
        blk_k = Wk_g[:, h * DH : (h + 1) * DH]
        wq[h] = (blk_q[:, perm] * scale).astype(npdt)
        wk[h] = blk_k[:, perm].astype(npdt)

    cos, sin = _rope_tables(t)
    return {
        "xt": np.ascontiguousarray(x_b.T).astype(npdt),
        "wq": wq,
        "wk": wk,
        "wv": np.ascontiguousarray(
            Wv_g.astype(npdt).reshape(d, vg, vgw).transpose(1, 0, 2)
        ),
        "wo": Wo_g.astype(npdt),
        "cos": cos,
        "sin": sin,
        "tri": _tri(),
        "ones": np.ones((P, P), npdt),
    }


def make_in_maps(inputs, npdt=np.float16):
    x, Wq, Wk, Wv, Wo = (
        np.asarray(inputs["x"]),
        np.asarray(inputs["Wq"]),
        np.asarray(inputs["Wk"]),
        np.asarray(inputs["Wv"]),
        np.asarray(inputs["Wo"]),
    )
    in_maps = []
    for c in range(NCORES):
        b, g = c // 2, c % 2
        cols = slice(g * HPC * DH, (g + 1) * HPC * DH)
        in_maps.append(
            prep_core_inputs(
                x[b], Wq[:, cols], Wk[:, cols], Wv[:, cols], Wo[cols, :], npdt=npdt
            )
        )
    return in_maps


def _build_sharded(nc, n_cores=NCORES):
    """Build a reusable jitted 8-core executable (bass2jax multi-core path,
    without output donation so it can be re-invoked for timing)."""
    import jax
    from jax.experimental.shard_map import shard_map
    from jax.sharding import Mesh, NamedSharding, PartitionSpec

    from concourse import bass2jax

    bass2jax.install_neuronx_cc_hook()
    partition_name = nc.partition_id_tensor.name if nc.partition_id_tensor else None
    in_names, out_names, out_avals, zero_outs = [], [], [], []
    for alloc in nc.m.functions[0].allocations:
        if not isinstance(alloc, mybir.MemoryLocationSet):
            continue
        name = alloc.memorylocations[0].name
        if alloc.kind == "ExternalInput":
            if name != partition_name:
                in_names.append(name)
        elif alloc.kind == "ExternalOutput":
            out_names.append(name)
            shape = tuple(alloc.tensor_shape)
            dtype = mybir.dt.np(alloc.dtype)
            out_avals.append(jax.core.ShapedArray(shape, dtype))
            zero_outs.append(np.zeros(shape, dtype))
    n_params = len(in_names)
    all_names = in_names + out_names
    if partition_name is not None:
        all_names = all_names + [partition_name]

    def _body(*args):
        operands = list(args)
        if partition_name is not None:
            operands.append(bass2jax.partition_id_tensor())
        outs = bass2jax._bass_exec_p.bind(
            *operands,
            out_avals=tuple(out_avals),
            in_names=tuple(all_names),
            out_names=tuple(out_names),
            lowering_input_output_aliases=(),
            sim_require_finite=True,
            sim_require_nnan=True,
            nc=nc,
        )
        return tuple(outs)

    def _chain(n):
        def f(*args):
            outs = _body(*args)
            for _ in range(n - 1):
                # 0-valued data dependency on the previous execution's first
                # output forces sequential NEFF executions on-device
                dep = (outs[0].ravel()[0] * 0).astype(args[0].dtype)
                outs = _body(args[0] + dep, *args[1:])
            return outs

        return f

    devices = jax.devices()[:n_cores]
    mesh = Mesh(np.asarray(devices), ("core",))
    in_specs = (PartitionSpec("core"),) * (n_params + len(out_names))
    out_specs = (PartitionSpec("core"),) * len(out_names)

    def _jit(body):
        return jax.jit(
            shard_map(
                body, mesh=mesh, in_specs=in_specs, out_specs=out_specs, check_rep=False
            ),
            keep_unused=True,
        )

    fn = _jit(_body)
    sharding = NamedSharding(mesh, PartitionSpec("core"))
    return fn, _jit, _chain, sharding, in_names, out_names, out_avals, zero_outs


def run_timed(nc, in_maps, reps=6, chain=0, n_cores=NCORES):
    """Run on all cores; return (per-core results, per-exec device ns)."""
    import time

    import jax

    fn, _jit, _chain, sharding, in_names, out_names, out_avals, zero_outs = (
        _build_sharded(nc, n_cores)
    )
    concat_in = [
        np.concatenate([np.asarray(in_maps[c][n]) for c in range(n_cores)], axis=0)
        for n in in_names
    ]
    concat_zeros = [
        np.zeros((n_cores * z.shape[0], *z.shape[1:]), z.dtype) for z in zero_outs
    ]
    dev_in = [jax.device_put(a, sharding) for a in concat_in]
    dev_zeros = [jax.device_put(a, sharding) for a in concat_zeros]
    out = jax.block_until_ready(fn(*dev_in, *dev_zeros))

    def _time(f):
        ts = []
        for _ in range(reps):
            t0 = time.perf_counter()
            jax.block_until_ready(f(*dev_in, *dev_zeros))
            ts.append(time.perf_counter() - t0)
        print("rep times (ms):", [f"{x * 1e3:.2f}" for x in ts])
        return min(ts)

    exec_ns = None
    if chain and chain > 1:
        fnc = _jit(_chain(chain))
        jax.block_until_ready(fnc(*dev_in, *dev_zeros))  # compile
        t1 = _time(fn)
        tn = _time(fnc)
        exec_ns = int((tn - t1) / (chain - 1) * 1e9)
        print(f"single call: {t1 * 1e3:.2f} ms, chain-{chain}: {tn * 1e3:.2f} ms")
    else:
        exec_ns = int(_time(fn) * 1e9)
    results = [
        {
            name: np.asarray(out[i]).reshape(n_cores, *out_avals[i].shape)[c]
            for i, name in enumerate(out_names)
        }
        for c in range(n_cores)
    ]
    return results, exec_ns


def kernel(x, Wq, Wk, Wv, Wo):
    nc = build()
    in_maps = make_in_maps({"x": x, "Wq": Wq, "Wk": Wk, "Wv": Wv, "Wo": Wo})
    results = run_bass_kernel_spmd(nc, in_maps, core_ids=list(range(NCORES))).results
    out = np.empty((B, T, D), np.float32)
    for b in range(B):
        out[b] = (
            results[2 * b]["y"].astype(np.float32)
            + results[2 * b + 1]["y"].astype(np.float32)
        ).T
    return out


# revision 48
# speedup vs baseline: 1.0006x; 1.0006x over previous
"""Causal multi-head attention with RoPE for TRN2 (Bass/Tile), 8 NeuronCores.

Problem: y = (softmax(causal(rope(x@Wq) @ rope(x@Wk)^T / sqrt(dh))) @ (x@Wv)) @ Wo
  B=4, T=2048, D=2048, H=16 heads, dh=128, fp32 I/O.

Sharding: 4-way batch data-parallel x 2-way head tensor-parallel.
  Core c: batch b = c//2, head group g = c%2 (heads 8g..8g+7).
  Each core computes a partial y[b] (its 8 heads' contribution through Wo);
  the host sums the two partials per batch.

Head-pipelined schedule: the Act-engine exp stream of head h's attention is
hidden under the PE-bound Q/K projection of head h+1 (and under the Wo output
projection for the last head) by interleaving instruction emission. Q/K never
round-trip through DRAM: per-head [128, t] tiles live in SBUF with 2-deep
head rotation. V is projected in a prologue (PE-bound, nothing to hide yet).

All matmuls run in fp16 (full PE rate; fp32 PSUM accumulation), transposed
layouts throughout (no on-chip transposes):
  - Projections contract over D with x^T resident in SBUF: Q^T/K^T produced as
    [dh, t]; V as [t, dh].
  - S^T[k, q] = (K^T chunk) stationary against Q^T moving; exp(S^T) is
    directly the moving operand of the P@V matmul -> O^T [dh, q], which is
    directly the moving operand of the Wo projection.
Causal diagonal 128-blocks are computed at partial width (S, exp, PV and the
rowsum all skip fully-masked columns); only the triangular first 128 valid
columns of each diagonal chunk need a mask multiply.
Softmax: no max subtraction (logits are O(+-6), exp is fp32-safe); the
denominator comes from a ones-vector matmul over DVE pair-tree partial sums;
each quad's ones-matmul is deferred one block so PE never waits on the tree.
RoPE: pair partners are pre-permuted into partition halves (even dh dims ->
partitions 0..63, odd -> 64..127) via a host-side column permutation of Wq/Wk,
making rotate-half a uniform +-64-partition shift on chip.
"""

import numpy as np

import concourse.bass as bass
import concourse.tile as tile
from concourse import bacc, mybir
from concourse.bass import ts
from concourse.bass_utils import run_bass_kernel_spmd

B, T, D = 4, 2048, 2048
H = 16
DH = 128
THETA = 10000.0
NCORES = 8
HPC = H // 2  # heads per core (2-way head TP)
P = 128
TQ = 512  # q-tile width
F16 = mybir.dt.float16
F32 = mybir.dt.float32


def build(t=T, d=D, hpc=HPC, reps=1, mmdt=None, trunc=None, mut=None):
    """Build the per-core Bass program (same program on all cores).

    reps>1 wraps the whole computation in a hardware loop (timing builds).
    trunc: None=full, "P"=prologue only, k (int)=prologue + k slots,
    "A"=everything except the outproj blocks (timing bisection builds).
    mut: timing-experiment mutations (break numerics): "noexp" (DVE copy
    instead of Act exp), "noil" (no proj/attn interleave), "noattn"/"noproj"
    (slots emit only one side), "norow" (skip rowsum tree+ones+recip).
    """
    nc = bacc.Bacc("TRN2", target_bir_lowering=False, debug=False)
    MMDT = mmdt or F16
    dc = d // P  # contraction chunks for projections
    tc_n = t // P  # token chunks (k-chunks in attention)
    ntq = t // TQ  # q tiles
    nnt = d // P  # output-projection row chunks
    vg = 2  # V head-groups (4 heads each, N=512)
    vgh = hpc // vg
    vgw = vgh * DH

    xt = nc.dram_tensor("xt", [d, t], MMDT, kind="ExternalInput").ap()
    wq = nc.dram_tensor("wq", [hpc, d, DH], MMDT, kind="ExternalInput").ap()
    wk = nc.dram_tensor("wk", [hpc, d, DH], MMDT, kind="ExternalInput").ap()
    wv = nc.dram_tensor("wv", [vg, d, vgw], MMDT, kind="ExternalInput").ap()
    wo = nc.dram_tensor("wo", [hpc * DH, d], MMDT, kind="ExternalInput").ap()
    cos = nc.dram_tensor("cos", [P, t], F16, kind="ExternalInput").ap()
    sin = nc.dram_tensor("sin", [P, t], F16, kind="ExternalInput").ap()
    tri = nc.dram_tensor("tri", [P, P], MMDT, kind="ExternalInput").ap()
    ones = nc.dram_tensor("ones", [P, P], MMDT, kind="ExternalInput").ap()
    y = nc.dram_tensor("y", [d, t], F16, kind="ExternalOutput").ap()
    wo_r = wo.rearrange("(h p) n -> p h n", p=P)

    with tile.TileContext(nc) as tc:
        import contextlib

        from contextlib import ExitStack

        loop_cm = tc.For_i(0, reps, 1) if reps > 1 else contextlib.nullcontext()
        with loop_cm, ExitStack() as stk:
            order = ["const", "xt", "qk", "vpool"]
            pools = {}
            for pname in order:
                pools[pname] = stk.enter_context(
                    tc.tile_pool(name=pname, bufs=2 if pname == "qk" else 1)
                )
            vp, constp, xtp, qkp = (
                pools["vpool"], pools["const"], pools["xt"], pools["qk"]
            )
            wsp = stk.enter_context(tc.tile_pool(name="wstream", bufs=2))
            rp_ = stk.enter_context(tc.tile_pool(name="rope", bufs=1))
            esp = stk.enter_context(tc.tile_pool(name="exps", bufs=2))
            trp = stk.enter_context(tc.tile_pool(name="tree", bufs=2))
            smp = stk.enter_context(tc.tile_pool(name="small", bufs=2))
            qkpsp = stk.enter_context(
                tc.tile_pool(name="qkpsum", bufs=1, space="PSUM"))
            spsp = stk.enter_context(
                tc.tile_pool(name="spsum", bufs=4, space="PSUM"))
            opsp = stk.enter_context(
                tc.tile_pool(name="opsum", bufs=1, space="PSUM"))
            rpsp = stk.enter_context(
                tc.tile_pool(name="rpsum", bufs=1, space="PSUM"))
            v_sb = vp.tile([P, tc_n, vg * vgw], MMDT, tag="v", name="v")
            cos_sb = constp.tile([P, t], F16, tag="cos")
            sin_sb = constp.tile([P, t], F16, tag="sin")
            tri_sb = constp.tile([P, P], MMDT, tag="tri")
            ones_sb = constp.tile([P, P], MMDT, tag="ones")
            xt_sb = xtp.tile([P, dc, t], MMDT, tag="xt")

            def load_consts():
                nc.sync.dma_start(cos_sb[:], cos)
                nc.sync.dma_start(sin_sb[:], sin)
                nc.sync.dma_start(tri_sb[:], tri)
                nc.sync.dma_start(ones_sb[:], ones)

            def load_x(q4s):
                for q4 in q4s:
                    for c in range(dc):
                        nc.sync.dma_start(
                            xt_sb[:, c, ts(q4, TQ)], xt[ts(c, P), ts(q4, TQ)]
                        )

            def load_w(h):
                out = []
                for name, w_ap in (("q", wq[h]), ("k", wk[h])):
                    w_sb = wsp.tile([P, dc, DH], MMDT, tag=f"w{name}")
                    nc.sync.dma_start(
                        w_sb[:], w_ap.rearrange("(c p) m -> p c m", p=P)
                    )
                    out.append(w_sb)
                return out

            def rope_pair_drain(pq, jt, qk_sb):
                # rope on the paired q||k psum tile [P, 2, TQ] in 4 DVE ops:
                # out = pq*cos + swap(pq)*nsin, nsin's top half pre-negated
                # host-side so rotate-half is two plain muls with crossed
                # partition halves (scalar_tensor_tensor is pathologically
                # slow on hw and avoided; op COUNT matters - each DVE op
                # carries ~300ns fixed overhead).
                def bc(tab, p0, p1):
                    return tab[p0:p1, ts(jt, TQ)].unsqueeze(1).broadcast_to(
                        [p1 - p0, 2, TQ]
                    )

                rot = rp_.tile([P, 2, TQ], F16, tag="rot")
                nc.vector.tensor_mul(rot[0:64], pq[64:128], bc(sin_sb, 0, 64))
                nc.vector.tensor_mul(rot[64:128], pq[0:64], bc(sin_sb, 64, 128))
                t1 = rp_.tile([P, 2, TQ], F16, tag="t1")
                nc.vector.tensor_mul(t1[:], pq[:], bc(cos_sb, 0, 128))
                nc.vector.tensor_add(qk_sb[:, jt], t1[:], rot[:])

            def proj_qk_blocks(w_pair, qk_sb):
                """16 closures per head (4 per jt: two half-chains each for q
                and k, rope after the k chain), projecting into the paired
                psum tile. Half-chain granularity (~1.8us PE) lets the merger
                put PE work between every attn s-burst and pv-burst."""
                blocks = []
                hc = dc // 2
                for jt in range(ntq):
                    pq_box = [None]

                    def piece(qk, half, jt=jt, pq_box=pq_box):
                        if qk == 0 and half == 0:
                            pq_box[0] = qkpsp.tile(
                                [P, 2, TQ], F32, tag="pq", name="pq"
                            )
                        pq = pq_box[0]
                        for c in range(half * hc, (half + 1) * hc):
                            nc.tensor.matmul(
                                pq[:, qk],
                                w_pair[qk][:, c, :],
                                xt_sb[:, c, ts(jt, TQ)],
                                start=(c == 0),
                                stop=(c == dc - 1),
                            )
                        if qk == 1 and half == 1:
                            rope_pair_drain(pq, jt, qk_sb)

                    for qk in range(2):
                        for half in range(2):
                            blocks.append(
                                lambda f=piece, qk=qk, half=half: f(qk, half)
                            )
                return blocks

            def load_wv(wvp):
                # chunk-granular DMAs interleaved with x's first q-tile so the
                # first V-projection matmul can start after ~2 chunks arrive
                wv_all = [
                    wvp.tile([P, dc, vgw], MMDT, tag=f"wv{g}", name=f"wv{g}")
                    for g in range(vg)
                ]
                wv_r = [wv[g].rearrange("(c p) m -> p c m", p=P) for g in range(vg)]
                for c in range(dc):
                    nc.sync.dma_start(
                        xt_sb[:, c, ts(0, TQ)], xt[ts(c, P), ts(0, TQ)]
                    )
                    for g in range(vg):
                        nc.sync.dma_start(wv_all[g][:, c, :], wv_r[g][:, c, :])
                return wv_all

            def proj_v(wv_all):
                # uses the (idle in prologue) sp psum pool: 4 rotating banks
                # so each group's drain overlaps later chains
                for tt in range(tc_n):
                    pvs = [spsp.tile([P, vgw], F32, tag="sp", name=f"pv{g}")
                           for g in range(vg)]
                    for g in range(vg):
                        for c in range(dc):
                            nc.tensor.matmul(
                                pvs[g][:],
                                xt_sb[:, c, ts(tt, P)],
                                wv_all[g][:, c, :],
                                start=(c == 0),
                                stop=(c == dc - 1),
                            )
                        nc.vector.tensor_copy(
                            v_sb[:, tt, g * vgw : (g + 1) * vgw], pvs[g][:]
                        )

            # ---------------- attention for one head ----------------
            def attn_blocks(h, qk_sb, aot_sb):
                """Returns [(kind, closure)] with kind in {"s","pv","end"};
                the merger places a proj/outproj piece between every s-burst
                and its pv-burst so the Act exp latency is always covered by
                PE work."""
                g, hh = divmod(h, vgh)

                def kslice(c):
                    return qk_sb[:, c // 4, 1, (c % 4) * P : (c % 4 + 1) * P]

                blocks = []
                for jt in range(ntq):
                    nch = (jt + 1) * (TQ // P)
                    nquad = nch // 4
                    state = {"sps": {}, "esq": None, "pend": [],
                             "op": None, "rp": None}

                    def s_burst(q, jt=jt, state=state):
                        if q == 0:
                            state["op"] = opsp.tile([P, TQ], F32, tag="op", name="op")
                            state["rp"] = rpsp.tile([P, TQ], F32, tag="rp", name="rp")
                        for c in range(4 * q, 4 * q + 4):
                            o = c - jt * (TQ // P)
                            w0 = max(0, o) * P  # first valid column
                            sp = spsp.tile([P, TQ], F32, tag="sp", name="sp")
                            nc.tensor.matmul(
                                sp[:, w0:TQ],
                                kslice(c),
                                qk_sb[:, jt, 0, w0:TQ],
                                start=True,
                                stop=True,
                            )
                            state["sps"][c] = (sp, w0)

                    def pv_burst(q, nquad, jt=jt, state=state, nch=nch):
                        # exp + mask + PV for chunks of quad q into a single
                        # [P, 4, TQ] quad tile (fewer, larger DVE tree ops);
                        # quad q's ones-matmul is deferred into the next block
                        # so PE never waits on the tree.
                        cs = list(range(4 * q, 4 * q + 4))
                        esq = esp.tile([P, 4, TQ], MMDT, tag="esq")
                        diag = cs[0] - jt * (TQ // P) >= 0
                        for i, c in enumerate(cs):
                            sp, w0 = state["sps"].pop(c)
                            if mut == "noexp":
                                nc.vector.tensor_copy(esq[:, i, w0:TQ], sp[:, w0:TQ])
                            else:
                                nc.scalar.activation(
                                    esq[:, i, w0:TQ], sp[:, w0:TQ],
                                    mybir.ActivationFunctionType.Exp,
                                )
                            if diag:
                                # triangular mask on the first valid 128 cols
                                nc.vector.tensor_mul(
                                    esq[:, i, w0 : w0 + P],
                                    esq[:, i, w0 : w0 + P],
                                    tri_sb[:],
                                )
                            nc.tensor.matmul(
                                state["op"][:, w0:TQ],
                                v_sb[:, c, g * vgw + hh * DH : g * vgw + (hh + 1) * DH],
                                esq[:, i, w0:TQ],
                                start=(c == 0),
                                stop=(c == nch - 1),
                            )
                        if mut == "norow":
                            return
                        # rowsum tree (off PE critical path)
                        if diag:
                            # cascade partial widths into slice 0
                            for i in range(1, 4):
                                w0 = i * P
                                nc.vector.tensor_add(
                                    esq[:, 0, w0:TQ], esq[:, 0, w0:TQ],
                                    esq[:, i, w0:TQ],
                                )
                            equad = esq[:, 0, :]
                        else:
                            th = trp.tile([P, 2, TQ], MMDT, tag="th")
                            nc.vector.tensor_add(
                                th[:], esq[:, 0:2, :], esq[:, 2:4, :]
                            )
                            eq = trp.tile([P, TQ], MMDT, tag="eq")
                            nc.vector.tensor_add(eq[:], th[:, 0, :], th[:, 1, :])
                            equad = eq[:]

                        if state["pend"]:
                            state["pend"].pop(0)()

                        def ones_mm(q=q, equad=equad, nquad=nquad, state=state):
                            nc.tensor.matmul(
                                state["rp"][:],
                                ones_sb[:],
                                equad,
                                start=(q == 0),
                                stop=(q == nquad - 1),
                            )
                        state["pend"].append(ones_mm)

                    def jt_end(h=h, jt=jt, state=state, aot_sb=aot_sb):
                        if mut == "norow":
                            nc.vector.tensor_copy(
                                aot_sb[:, h, ts(jt, TQ)], state["op"][:]
                            )
                            return
                        while state["pend"]:
                            state["pend"].pop(0)()
                        rs = smp.tile([P, TQ], F32, tag="rs")
                        nc.vector.reciprocal(rs[:], state["rp"][:])
                        nc.vector.tensor_mul(
                            aot_sb[:, h, ts(jt, TQ)], state["op"][:], rs[:]
                        )

                    for q in range(nquad):
                        blocks.append(("s", lambda f=s_burst, q=q: f(q)))
                        blocks.append(("pv", lambda f=pv_burst, q=q, nq=nquad: f(q, nq)))
                    blocks.append(("end", jt_end))
                return blocks

            y_r = y.rearrange("(c p) t -> p c t", p=P)

            def outproj_blocks(jt, aot_sb, wop, cdp):
                """Output projection columns tq=jt: nnt/2 block closures, one
                nt-PAIR each (two 8-head accumulation chains into the paired
                pq psum tile - idle in the final slot - one paired drain +
                DMA); wo streamed by nt with prefetch (re-streamed each jt:
                4 MB x 4, fully hidden)."""
                wo_tiles = {}

                def load(nt):
                    wo_nt = wop.tile([P, hpc, P], MMDT, tag="wo")
                    nc.sync.dma_start(wo_nt[:], wo_r[:, :, ts(nt, P)])
                    wo_tiles[nt] = wo_nt

                def blk(nt, jt=jt):
                    if nt == 0:
                        for i in range(6):
                            load(i)
                    yp = qkpsp.tile([P, 2, TQ], F32, tag="pq", name="yp")
                    for a in range(2):
                        wo_nt = wo_tiles.pop(nt + a)
                        for h in range(hpc):
                            nc.tensor.matmul(
                                yp[:, a],
                                wo_nt[:, h, :],
                                aot_sb[:, h, ts(jt, TQ)],
                                start=(h == 0),
                                stop=(h == hpc - 1),
                            )
                        if nt + a + 6 < nnt:
                            load(nt + a + 6)
                    ytile = cdp.tile([P, 2, TQ], F16, tag="ytile")
                    nc.vector.tensor_copy(ytile[:], yp[:])
                    nc.sync.dma_start(
                        y_r[:, nt : nt + 2, ts(jt, TQ)], ytile[:]
                    )

                return [lambda nt=nt: blk(nt) for nt in range(0, nnt, 2)]

            def interleave(primary, secondary):
                """Emit tagged primary (attn) blocks with secondary (proj /
                outproj) pieces spliced so that one piece lands between every
                s-burst and its pv-burst (covering the Act exp latency with
                PE work), surplus drained proportionally at jt ends."""
                ns_ = len(secondary)
                npv = sum(1 for k, _ in primary if k == "pv") or 1
                si = 0
                pvi = 0
                for kind, blk in primary:
                    if kind == "pv":
                        pvi += 1
                        want = min(ns_, -(-pvi * ns_ // npv))  # ceil
                        while si < want:
                            secondary[si]()
                            si += 1
                    blk()
                while si < ns_:
                    secondary[si]()
                    si += 1

            # ======================= schedule =======================
            # prologue: x/V (PE-bound, nothing to hide) + head-0 Q/K
            wvp_cm = tc.tile_pool(name="wvp", bufs=1)
            wvp = wvp_cm.__enter__()
            wv_all = load_wv(wvp)
            load_consts()
            load_x([1, 2, 3])
            w_cur = load_w(0)
            proj_v(wv_all)
            cur_qk = qkp.tile([P, ntq, 2, TQ], MMDT, tag="qk")
            for blk in proj_qk_blocks(w_cur, cur_qk):
                blk()
            wvp_cm.__exit__(None, None, None)

            # aot + slot-7 pools open after wv's SBUF is released
            aot_cm = tc.tile_pool(name="aot", bufs=1)
            aotp = aot_cm.__enter__()
            wop_cm = tc.tile_pool(name="wostream", bufs=6)
            wop = wop_cm.__enter__()
            cdp_cm = tc.tile_pool(name="cdrain", bufs=2)
            cdp = cdp_cm.__enter__()
            aot_sb = aotp.tile([P, hpc, t], MMDT, tag="aot")

            # slots 1..7: attn(h-1) interleaved with proj_qk(h)
            nslot = hpc if trunc in (None, "A") else (1 if trunc == "P" else trunc + 1)
            w_next = load_w(1)
            for h in range(1, nslot):
                w_cur = w_next
                nxt_qk = qkp.tile([P, ntq, 2, TQ], MMDT, tag="qk")
                pb = proj_qk_blocks(w_cur, nxt_qk)
                if h + 1 < hpc:
                    w_next = load_w(h + 1)
                ab = attn_blocks(h - 1, cur_qk, aot_sb)
                if mut == "noattn":
                    ab = []
                elif mut == "noproj":
                    pb = []
                if mut == "noil":
                    for _, blk in ab:
                        blk()
                    for blk in pb:
                        blk()
                else:
                    interleave(ab, pb)
                cur_qk = nxt_qk

            # final slot: attn(7), with outproj(jt-1) interleaved into the
            # attn jt group so the jt_end -> outproj dependency wait is hidden
            if trunc in (None, "A"):
                ab = attn_blocks(hpc - 1, cur_qk, aot_sb)
                groups = []
                abi = 0
                for jt in range(ntq):
                    n = 2 * (jt + 1) + 1
                    groups.append(ab[abi : abi + n])
                    abi += n
                assert abi == len(ab)
                for _, blk in groups[0]:
                    blk()
                for jt in range(1, ntq):
                    interleave(
                        groups[jt],
                        outproj_blocks(jt - 1, aot_sb, wop, cdp)
                        if trunc is None else [],
                    )
                if trunc is None:
                    for blk in outproj_blocks(ntq - 1, aot_sb, wop, cdp):
                        blk()

            cdp_cm.__exit__(None, None, None)
            wop_cm.__exit__(None, None, None)
            aot_cm.__exit__(None, None, None)

    nc.compile()
    return nc


def _rope_tables(t=T):
    """cos/sin in transposed+permuted layout [128, t] (fp16).

    Partition p < 64 holds dh dim 2p (even), p >= 64 holds dh dim 2(p-64)+1;
    pair (2i, 2i+1) shares inv_freq[i], so row p uses inv_freq[p % 64].
    """
    inv_freq = 1.0 / (THETA ** (np.arange(0, DH, 2, dtype=np.float64) / DH))  # [64]
    pos = np.arange(t, dtype=np.float64)
    freqs = pos[None, :] * inv_freq[np.arange(P) % 64][:, None]  # [128, t]
    sin = np.sin(freqs)
    sin[:64] = -sin[:64]  # rotate-half sign baked into the table's top half
    return (
        np.cos(freqs).astype(np.float16),
        sin.astype(np.float16),
    )


def _perm():
    """Within-head dh permutation: even dims first, then odd dims."""
    return np.concatenate([np.arange(0, DH, 2), np.arange(1, DH, 2)])


def _tri():
    """tri[dk, dq] = 1 if dk <= dq else 0 (within-chunk causal triangle)."""
    dk = np.arange(P)[:, None]
    dq = np.arange(P)[None, :]
    return (dk <= dq).astype(np.float16)


def prep_core_inputs(x_b, Wq_g, Wk_g, Wv_g, Wo_g, t=T, hpc=HPC, npdt=np.float16):
    """Host-side input prep for one core.

    x_b: [t, D] (this core's batch); W*_g: this core's head-group slices
    (Wq/Wk/Wv: [D, hpc*DH] columns, Wo: [hpc*DH, D] rows).
    """
    d = x_b.shape[1]
    perm = _perm()
    scale = 1.0 / np.sqrt(DH)
    vg = 2
    vgw = (hpc // vg) * DH

    wq = np.empty((hpc, d, DH), npdt)
    wk = np.empty((hpc, d, DH), npdt)
    for h in range(hpc):
        blk_q = Wq_g[:, h * DH : (h + 1) * DH]
        blk_k = Wk_g[:, h * DH : (h + 1) * DH]
        wq[h] = (blk_q[:, perm] * scale).astype(npdt)
        wk[h] = blk_k[:, perm].astype(npdt)

    cos, sin = _rope_tables(t)
    return {
        "xt": np.ascontiguousarray(x_b.T).astype(npdt),
        "wq": wq,
        "wk": wk,
        "wv": np.ascontiguousarray(
            Wv_g.astype(npdt).reshape(d, vg, vgw).transpose(1, 0, 2)
        ),
        "wo": Wo_g.astype(npdt),
        "cos": cos,
        "sin": sin,
        "tri": _tri(),
        "ones": np.ones((P, P), npdt),
    }


def make_in_maps(inputs, npdt=np.float16):
    x, Wq, Wk, Wv, Wo = (
        np.asarray(inputs["x"]),
        np.asarray(inputs["Wq"]),
        np.asarray(inputs["Wk"]),
        np.asarray(inputs["Wv"]),
        np.asarray(inputs["Wo"]),
    )
    in_maps = []
    for c in range(NCORES):
        b, g = c // 2, c % 2
        cols = slice(g * HPC * DH, (g + 1) * HPC * DH)
        in_maps.append(
            prep_core_inputs(
                x[b], Wq[:, cols], Wk[:, cols], Wv[:, cols], Wo[cols, :], npdt=npdt
            )
        )
    return in_maps


def _build_sharded(nc, n_cores=NCORES):
    """Build a reusable jitted 8-core executable (bass2jax multi-core path,
    without output donation so it can be re-invoked for timing)."""
    import jax
    from jax.experimental.shard_map import shard_map
    from jax.sharding import Mesh, NamedSharding, PartitionSpec

    from concourse import bass2jax

    bass2jax.install_neuronx_cc_hook()
    partition_name = nc.partition_id_tensor.name if nc.partition_id_tensor else None
    in_names, out_names, out_avals, zero_outs = [], [], [], []
    for alloc in nc.m.functions[0].allocations:
        if not isinstance(alloc, mybir.MemoryLocationSet):
            continue
        name = alloc.memorylocations[0].name
        if alloc.kind == "ExternalInput":
            if name != partition_name:
                in_names.append(name)
        elif alloc.kind == "ExternalOutput":
            out_names.append(name)
            shape = tuple(alloc.tensor_shape)
            dtype = mybir.dt.np(alloc.dtype)
            out_avals.append(jax.core.ShapedArray(shape, dtype))
            zero_outs.append(np.zeros(shape, dtype))
    n_params = len(in_names)
    all_names = in_names + out_names
    if partition_name is not None:
        all_names = all_names + [partition_name]

    def _body(*args):
        operands = list(args)
        if partition_name is not None:
            operands.append(bass2jax.partition_id_tensor())
        outs = bass2jax._bass_exec_p.bind(
            *operands,
            out_avals=tuple(out_avals),
            in_names=tuple(all_names),
            out_names=tuple(out_names),
            lowering_input_output_aliases=(),
            sim_require_finite=True,
            sim_require_nnan=True,
            nc=nc,
        )
        return tuple(outs)

    def _chain(n):
        def f(*args):
            outs = _body(*args)
            for _ in range(n - 1):
                # 0-valued data dependency on the previous execution's first
                # output forces sequential NEFF executions on-device
                dep = (outs[0].ravel()[0] * 0).astype(args[0].dtype)
                outs = _body(args[0] + dep, *args[1:])
            return outs

        return f

    devices = jax.devices()[:n_cores]
    mesh = Mesh(np.asarray(devices), ("core",))
    in_specs = (PartitionSpec("core"),) * (n_params + len(out_names))
    out_specs = (PartitionSpec("core"),) * len(out_names)

    def _jit(body):
        return jax.jit(
            shard_map(
                body, mesh=mesh, in_specs=in_specs, out_specs=out_specs, check_rep=False
            ),
            keep_unused=True,
        )

    fn = _jit(_body)
    sharding = NamedSharding(mesh, PartitionSpec("core"))
    return fn, _jit, _chain, sharding, in_names, out_names, out_avals, zero_outs


def run_timed(nc, in_maps, reps=6, chain=0, n_cores=NCORES):
    """Run on all cores; return (per-core results, per-exec device ns)."""
    import time

    import jax

    fn, _jit, _chain, sharding, in_names, out_names, out_avals, zero_outs = (
        _build_sharded(nc, n_cores)
    )
    concat_in = [
        np.concatenate([np.asarray(in_maps[c][n]) for c in range(n_cores)], axis=0)
        for n in in_names
    ]
    concat_zeros = [
        np.zeros((n_cores * z.shape[0], *z.shape[1:]), z.dtype) for z in zero_outs
    ]
    dev_in = [jax.device_put(a, sharding) for a in concat_in]
    dev_zeros = [jax.device_put(a, sharding) for a in concat_zeros]
    out = jax.block_until_ready(fn(*dev_in, *dev_zeros))

    def _time(f):
        ts = []
        for _ in range(reps):
            t0 = time.perf_counter()
            jax.block_until_ready(f(*dev_in, *dev_zeros))
            ts.append(time.perf_counter() - t0)
        print("rep times (ms):", [f"{x * 1e3:.2f}" for x in ts])
        return min(ts)

    exec_ns = None
    if chain and chain > 1:
        fnc = _jit(_chain(chain))
        jax.block_until_ready(fnc(*dev_in, *dev_zeros))  # compile
        t1 = _time(fn)
        tn = _time(fnc)
        exec_ns = int((tn - t1) / (chain - 1) * 1e9)
        print(f"single call: {t1 * 1e3:.2f} ms, chain-{chain}: {tn * 1e3:.2f} ms")
    else:
        exec_ns = int(_time(fn) * 1e9)
    results = [
        {
            name: np.asarray(out[i]).reshape(n_cores, *out_avals[i].shape)[c]
            for i, name in enumerate(out_names)
        }
        for c in range(n_cores)
    ]
    return results, exec_ns


def kernel(x, Wq, Wk, Wv, Wo):
    nc = build()
    in_maps = make_in_maps({"x": x, "Wq": Wq, "Wk": Wk, "Wv": Wv, "Wo": Wo})
    results = run_bass_kernel_spmd(nc, in_maps, core_ids=list(range(NCORES))).results
    out = np.empty((B, T, D), np.float32)
    for b in range(B):
        out[b] = (
            results[2 * b]["y"].astype(np.float32)
            + results[2 * b + 1]["y"].astype(np.float32)
        ).T
    return out


# revision 49
# speedup vs baseline: 1.0430x; 1.0423x over previous
"""Causal multi-head attention with RoPE for TRN2 (Bass/Tile), 8 NeuronCores.

Problem: y = (softmax(causal(rope(x@Wq) @ rope(x@Wk)^T / sqrt(dh))) @ (x@Wv)) @ Wo
  B=4, T=2048, D=2048, H=16 heads, dh=128, fp32 I/O.

Sharding: 4-way batch data-parallel x 2-way head tensor-parallel.
  Core c: batch b = c//2, head group g = c%2 (heads 8g..8g+7).
  Each core computes a partial y[b] (its 8 heads' contribution through Wo);
  the host sums the two partials per batch.

Head-pipelined schedule: the Act-engine exp stream of head h's attention is
hidden under the PE-bound Q/K projection of head h+1 (and under the Wo output
projection for the last head) by interleaving instruction emission. Q/K never
round-trip through DRAM: per-head [128, t] tiles live in SBUF with 2-deep
head rotation. V is projected in a prologue (PE-bound, nothing to hide yet).

All matmuls run in fp16 (full PE rate; fp32 PSUM accumulation), transposed
layouts throughout (no on-chip transposes):
  - Projections contract over D with x^T resident in SBUF: Q^T/K^T produced as
    [dh, t]; V as [t, dh].
  - S^T[k, q] = (K^T chunk) stationary against Q^T moving; exp(S^T) is
    directly the moving operand of the P@V matmul -> O^T [dh, q], which is
    directly the moving operand of the Wo projection.
Causal diagonal 128-blocks are computed at partial width (S, exp, PV and the
rowsum all skip fully-masked columns); only the triangular first 128 valid
columns of each diagonal chunk need a mask multiply.
Softmax: no max subtraction (logits are O(+-6), exp is fp32-safe); the
denominator comes from a ones-vector matmul over DVE pair-tree partial sums;
each quad's ones-matmul is deferred one block so PE never waits on the tree.
RoPE: pair partners are pre-permuted into partition halves (even dh dims ->
partitions 0..63, odd -> 64..127) via a host-side column permutation of Wq/Wk,
making rotate-half a uniform +-64-partition shift on chip.
"""

import numpy as np

import concourse.bass as bass
import concourse.tile as tile
from concourse import bacc, mybir
from concourse.bass import ts
from concourse.bass_utils import run_bass_kernel_spmd

B, T, D = 4, 2048, 2048
H = 16
DH = 128
THETA = 10000.0
NCORES = 8
HPC = H // 2  # heads per core (2-way head TP)
P = 128
TQ = 512  # q-tile width
F16 = mybir.dt.float16
F32 = mybir.dt.float32


def build(t=T, d=D, hpc=HPC, reps=1, mmdt=None, trunc=None, mut=None):
    """Build the per-core Bass program (same program on all cores).

    reps>1 wraps the whole computation in a hardware loop (timing builds).
    trunc: None=full, "P"=prologue only, k (int)=prologue + k slots,
    "A"=everything except the outproj blocks (timing bisection builds).
    mut: timing-experiment mutations (break numerics): "noexp" (DVE copy
    instead of Act exp), "noil" (no proj/attn interleave), "noattn"/"noproj"
    (slots emit only one side), "norow" (skip rowsum tree+ones+recip).
    """
    nc = bacc.Bacc("TRN2", target_bir_lowering=False, debug=False)
    MMDT = mmdt or F16
    dc = d // P  # contraction chunks for projections
    tc_n = t // P  # token chunks (k-chunks in attention)
    ntq = t // TQ  # q tiles
    nnt = d // P  # output-projection row chunks
    vg = 2  # V head-groups (4 heads each, N=512)
    vgh = hpc // vg
    vgw = vgh * DH

    xt = nc.dram_tensor("xt", [d, t], MMDT, kind="ExternalInput").ap()
    wq = nc.dram_tensor("wq", [hpc, d, DH], MMDT, kind="ExternalInput").ap()
    wk = nc.dram_tensor("wk", [hpc, d, DH], MMDT, kind="ExternalInput").ap()
    wv = nc.dram_tensor("wv", [vg, d, vgw], MMDT, kind="ExternalInput").ap()
    wo = nc.dram_tensor("wo", [hpc * DH, d], MMDT, kind="ExternalInput").ap()
    cos = nc.dram_tensor("cos", [P, t], F16, kind="ExternalInput").ap()
    sin = nc.dram_tensor("sin", [P, t], F16, kind="ExternalInput").ap()
    tri = nc.dram_tensor("tri", [P, P], MMDT, kind="ExternalInput").ap()
    ones = nc.dram_tensor("ones", [P, P], MMDT, kind="ExternalInput").ap()
    y = nc.dram_tensor("y", [d, t], F16, kind="ExternalOutput").ap()
    wo_r = wo.rearrange("(h p) n -> p h n", p=P)

    with tile.TileContext(nc) as tc:
        import contextlib

        from contextlib import ExitStack

        loop_cm = tc.For_i(0, reps, 1) if reps > 1 else contextlib.nullcontext()
        with loop_cm, ExitStack() as stk:
            order = ["const", "xt", "qk", "vpool"]
            pools = {}
            for pname in order:
                pools[pname] = stk.enter_context(
                    tc.tile_pool(name=pname, bufs=2 if pname == "qk" else 1)
                )
            vp, constp, xtp, qkp = (
                pools["vpool"], pools["const"], pools["xt"], pools["qk"]
            )
            wsp = stk.enter_context(tc.tile_pool(name="wstream", bufs=2))
            rp_ = stk.enter_context(tc.tile_pool(name="rope", bufs=1))
            esp = stk.enter_context(tc.tile_pool(name="exps", bufs=2))
            trp = stk.enter_context(tc.tile_pool(name="tree", bufs=2))
            smp = stk.enter_context(tc.tile_pool(name="small", bufs=2))
            qkpsp = stk.enter_context(
                tc.tile_pool(name="qkpsum", bufs=1, space="PSUM"))
            spsp = stk.enter_context(
                tc.tile_pool(name="spsum", bufs=4, space="PSUM"))
            opsp = stk.enter_context(
                tc.tile_pool(name="opsum", bufs=1, space="PSUM"))
            rpsp = stk.enter_context(
                tc.tile_pool(name="rpsum", bufs=1, space="PSUM"))
            v_sb = vp.tile([P, tc_n, vg * vgw], MMDT, tag="v", name="v")
            cos_sb = constp.tile([P, t], F16, tag="cos")
            sin_sb = constp.tile([P, t], F16, tag="sin")
            tri_sb = constp.tile([P, P], MMDT, tag="tri")
            ones_sb = constp.tile([P, P], MMDT, tag="ones")
            xt_sb = xtp.tile([P, dc, t], MMDT, tag="xt")

            def load_consts():
                nc.sync.dma_start(cos_sb[:], cos)
                nc.sync.dma_start(sin_sb[:], sin)
                nc.sync.dma_start(tri_sb[:], tri)
                nc.sync.dma_start(ones_sb[:], ones)

            def load_x(q4s):
                for q4 in q4s:
                    for c in range(dc):
                        nc.sync.dma_start(
                            xt_sb[:, c, ts(q4, TQ)], xt[ts(c, P), ts(q4, TQ)]
                        )

            def load_w(h):
                out = []
                for name, w_ap in (("q", wq[h]), ("k", wk[h])):
                    w_sb = wsp.tile([P, dc, DH], MMDT, tag=f"w{name}")
                    nc.sync.dma_start(
                        w_sb[:], w_ap.rearrange("(c p) m -> p c m", p=P)
                    )
                    out.append(w_sb)
                return out

            def rope_pair_drain(pq, jt, qk_sb):
                # rope on the paired q||k psum tile [P, 2, TQ] in 4 DVE ops:
                # out = pq*cos + swap(pq)*nsin, nsin's top half pre-negated
                # host-side so rotate-half is two plain muls with crossed
                # partition halves (scalar_tensor_tensor is pathologically
                # slow on hw and avoided; op COUNT matters - each DVE op
                # carries ~300ns fixed overhead).
                def bc(tab, p0, p1):
                    return tab[p0:p1, ts(jt, TQ)].unsqueeze(1).broadcast_to(
                        [p1 - p0, 2, TQ]
                    )

                rot = rp_.tile([P, 2, TQ], F16, tag="rot")
                nc.vector.tensor_mul(rot[0:64], pq[64:128], bc(sin_sb, 0, 64))
                nc.vector.tensor_mul(rot[64:128], pq[0:64], bc(sin_sb, 64, 128))
                t1 = rp_.tile([P, 2, TQ], F16, tag="t1")
                nc.vector.tensor_mul(t1[:], pq[:], bc(cos_sb, 0, 128))
                nc.vector.tensor_add(qk_sb[:, jt], t1[:], rot[:])

            def proj_qk_blocks(w_pair, qk_sb):
                """8 closures per head: per jt a q-chain piece and a k-chain +
                rope piece, projecting into the paired psum tile. (Finer
                half-chain pieces measured WORSE on hw - interrupting open
                accumulation chains more often costs more than the extra
                exp-latency cover buys.)"""
                blocks = []
                for jt in range(ntq):
                    pq_box = [None]

                    def blk_q(jt=jt, pq_box=pq_box):
                        pq_box[0] = qkpsp.tile([P, 2, TQ], F32, tag="pq", name="pq")
                        for c in range(dc):
                            nc.tensor.matmul(
                                pq_box[0][:, 0],
                                w_pair[0][:, c, :],
                                xt_sb[:, c, ts(jt, TQ)],
                                start=(c == 0),
                                stop=(c == dc - 1),
                            )

                    def blk_k(jt=jt, pq_box=pq_box):
                        pq = pq_box[0]
                        for c in range(dc):
                            nc.tensor.matmul(
                                pq[:, 1],
                                w_pair[1][:, c, :],
                                xt_sb[:, c, ts(jt, TQ)],
                                start=(c == 0),
                                stop=(c == dc - 1),
                            )
                        rope_pair_drain(pq, jt, qk_sb)

                    blocks.append(blk_q)
                    blocks.append(blk_k)
                return blocks

            def load_wv(wvp):
                # chunk-granular DMAs interleaved with x's first q-tile so the
                # first V-projection matmul can start after ~2 chunks arrive
                wv_all = [
                    wvp.tile([P, dc, vgw], MMDT, tag=f"wv{g}", name=f"wv{g}")
                    for g in range(vg)
                ]
                wv_r = [wv[g].rearrange("(c p) m -> p c m", p=P) for g in range(vg)]
                for c in range(dc):
                    nc.sync.dma_start(
                        xt_sb[:, c, ts(0, TQ)], xt[ts(c, P), ts(0, TQ)]
                    )
                    for g in range(vg):
                        nc.sync.dma_start(wv_all[g][:, c, :], wv_r[g][:, c, :])
                return wv_all

            def proj_v(wv_all):
                # uses the (idle in prologue) sp psum pool: 4 rotating banks
                # so each group's drain overlaps later chains
                for tt in range(tc_n):
                    pvs = [spsp.tile([P, vgw], F32, tag="sp", name=f"pv{g}")
                           for g in range(vg)]
                    for g in range(vg):
                        for c in range(dc):
                            nc.tensor.matmul(
                                pvs[g][:],
                                xt_sb[:, c, ts(tt, P)],
                                wv_all[g][:, c, :],
                                start=(c == 0),
                                stop=(c == dc - 1),
                            )
                        nc.vector.tensor_copy(
                            v_sb[:, tt, g * vgw : (g + 1) * vgw], pvs[g][:]
                        )

            # ---------------- attention for one head ----------------
            def attn_blocks(h, qk_sb, aot_sb):
                """Returns [(kind, closure)] with kind in {"s","pv","end"};
                the merger places a proj/outproj piece between every s-burst
                and its pv-burst so the Act exp latency is always covered by
                PE work."""
                g, hh = divmod(h, vgh)

                def kslice(c):
                    return qk_sb[:, c // 4, 1, (c % 4) * P : (c % 4 + 1) * P]

                blocks = []
                for jt in range(ntq):
                    nch = (jt + 1) * (TQ // P)
                    nquad = nch // 4
                    state = {"sps": {}, "esq": None, "pend": [],
                             "op": None, "rp": None}

                    def s_burst(q, jt=jt, state=state):
                        if q == 0:
                            state["op"] = opsp.tile([P, TQ], F32, tag="op", name="op")
                            state["rp"] = rpsp.tile([P, TQ], F32, tag="rp", name="rp")
                        for c in range(4 * q, 4 * q + 4):
                            o = c - jt * (TQ // P)
                            w0 = max(0, o) * P  # first valid column
                            sp = spsp.tile([P, TQ], F32, tag="sp", name="sp")
                            nc.tensor.matmul(
                                sp[:, w0:TQ],
                                kslice(c),
                                qk_sb[:, jt, 0, w0:TQ],
                                start=True,
                                stop=True,
                            )
                            state["sps"][c] = (sp, w0)

                    def pv_burst(q, nquad, jt=jt, state=state, nch=nch):
                        # exp + mask + PV for chunks of quad q into a single
                        # [P, 4, TQ] quad tile (fewer, larger DVE tree ops);
                        # quad q's ones-matmul is deferred into the next block
                        # so PE never waits on the tree.
                        cs = list(range(4 * q, 4 * q + 4))
                        esq = esp.tile([P, 4, TQ], MMDT, tag="esq")
                        diag = cs[0] - jt * (TQ // P) >= 0
                        for i, c in enumerate(cs):
                            sp, w0 = state["sps"].pop(c)
                            if mut == "noexp":
                                nc.vector.tensor_copy(esq[:, i, w0:TQ], sp[:, w0:TQ])
                            else:
                                nc.scalar.activation(
                                    esq[:, i, w0:TQ], sp[:, w0:TQ],
                                    mybir.ActivationFunctionType.Exp,
                                )
                            if diag:
                                # triangular mask on the first valid 128 cols
                                nc.vector.tensor_mul(
                                    esq[:, i, w0 : w0 + P],
                                    esq[:, i, w0 : w0 + P],
                                    tri_sb[:],
                                )
                            nc.tensor.matmul(
                                state["op"][:, w0:TQ],
                                v_sb[:, c, g * vgw + hh * DH : g * vgw + (hh + 1) * DH],
                                esq[:, i, w0:TQ],
                                start=(c == 0),
                                stop=(c == nch - 1),
                            )
                        if mut == "norow":
                            return
                        # rowsum tree (off PE critical path)
                        if diag:
                            # cascade partial widths into slice 0
                            for i in range(1, 4):
                                w0 = i * P
                                nc.vector.tensor_add(
                                    esq[:, 0, w0:TQ], esq[:, 0, w0:TQ],
                                    esq[:, i, w0:TQ],
                                )
                            equad = esq[:, 0, :]
                        else:
                            th = trp.tile([P, 2, TQ], MMDT, tag="th")
                            nc.vector.tensor_add(
                                th[:], esq[:, 0:2, :], esq[:, 2:4, :]
                            )
                            eq = trp.tile([P, TQ], MMDT, tag="eq")
                            nc.vector.tensor_add(eq[:], th[:, 0, :], th[:, 1, :])
                            equad = eq[:]

                        if state["pend"]:
                            state["pend"].pop(0)()

                        def ones_mm(q=q, equad=equad, nquad=nquad, state=state):
                            nc.tensor.matmul(
                                state["rp"][:],
                                ones_sb[:],
                                equad,
                                start=(q == 0),
                                stop=(q == nquad - 1),
                            )
                        state["pend"].append(ones_mm)

                    def jt_end(h=h, jt=jt, state=state, aot_sb=aot_sb):
                        if mut == "norow":
                            nc.vector.tensor_copy(
                                aot_sb[:, h, ts(jt, TQ)], state["op"][:]
                            )
                            return
                        while state["pend"]:
                            state["pend"].pop(0)()
                        rs = smp.tile([P, TQ], F32, tag="rs")
                        nc.vector.reciprocal(rs[:], state["rp"][:])
                        nc.vector.tensor_mul(
                            aot_sb[:, h, ts(jt, TQ)], state["op"][:], rs[:]
                        )

                    for q in range(nquad):
                        blocks.append(("s", lambda f=s_burst, q=q: f(q)))
                        blocks.append(("pv", lambda f=pv_burst, q=q, nq=nquad: f(q, nq)))
                    blocks.append(("end", jt_end))
                return blocks

            y_r = y.rearrange("(c p) t -> p c t", p=P)

            def outproj_blocks(jt, aot_sb, wop, cdp):
                """Output projection columns tq=jt: nnt/2 block closures, one
                nt-PAIR each (two 8-head accumulation chains into the paired
                pq psum tile - idle in the final slot - one paired drain +
                DMA); wo streamed by nt with prefetch (re-streamed each jt:
                4 MB x 4, fully hidden)."""
                wo_tiles = {}

                def load(nt):
                    wo_nt = wop.tile([P, hpc, P], MMDT, tag="wo")
                    nc.sync.dma_start(wo_nt[:], wo_r[:, :, ts(nt, P)])
                    wo_tiles[nt] = wo_nt

                def blk(nt, jt=jt):
                    if nt == 0:
                        for i in range(6):
                            load(i)
                    yp = qkpsp.tile([P, 2, TQ], F32, tag="pq", name="yp")
                    for a in range(2):
                        wo_nt = wo_tiles.pop(nt + a)
                        for h in range(hpc):
                            nc.tensor.matmul(
                                yp[:, a],
                                wo_nt[:, h, :],
                                aot_sb[:, h, ts(jt, TQ)],
                                start=(h == 0),
                                stop=(h == hpc - 1),
                            )
                        if nt + a + 6 < nnt:
                            load(nt + a + 6)
                    ytile = cdp.tile([P, 2, TQ], F16, tag="ytile")
                    nc.vector.tensor_copy(ytile[:], yp[:])
                    nc.sync.dma_start(
                        y_r[:, nt : nt + 2, ts(jt, TQ)], ytile[:]
                    )

                return [lambda nt=nt: blk(nt) for nt in range(0, nnt, 2)]

            def interleave(primary, secondary):
                """Emit tagged primary (attn) blocks with secondary (proj /
                outproj) pieces spliced so that one piece lands between every
                s-burst and its pv-burst (covering the Act exp latency with
                PE work), surplus drained proportionally at jt ends."""
                ns_ = len(secondary)
                npv = sum(1 for k, _ in primary if k == "pv") or 1
                si = 0
                pvi = 0
                for kind, blk in primary:
                    if kind == "pv":
                        pvi += 1
                        want = min(ns_, -(-pvi * ns_ // npv))  # ceil
                        while si < want:
                            secondary[si]()
                            si += 1
                    blk()
                while si < ns_:
                    secondary[si]()
                    si += 1

            # ======================= schedule =======================
            # prologue: x/V (PE-bound, nothing to hide) + head-0 Q/K
            wvp_cm = tc.tile_pool(name="wvp", bufs=1)
            wvp = wvp_cm.__enter__()
            wv_all = load_wv(wvp)
            load_consts()
            load_x([1, 2, 3])
            w_cur = load_w(0)
            proj_v(wv_all)
            cur_qk = qkp.tile([P, ntq, 2, TQ], MMDT, tag="qk")
            for blk in proj_qk_blocks(w_cur, cur_qk):
                blk()
            wvp_cm.__exit__(None, None, None)

            # aot + slot-7 pools open after wv's SBUF is released
            aot_cm = tc.tile_pool(name="aot", bufs=1)
            aotp = aot_cm.__enter__()
            wop_cm = tc.tile_pool(name="wostream", bufs=6)
            wop = wop_cm.__enter__()
            cdp_cm = tc.tile_pool(name="cdrain", bufs=2)
            cdp = cdp_cm.__enter__()
            aot_sb = aotp.tile([P, hpc, t], MMDT, tag="aot")

            # slots 1..7: attn(h-1) interleaved with proj_qk(h)
            nslot = hpc if trunc in (None, "A") else (1 if trunc == "P" else trunc + 1)
            w_next = load_w(1)
            for h in range(1, nslot):
                w_cur = w_next
                nxt_qk = qkp.tile([P, ntq, 2, TQ], MMDT, tag="qk")
                pb = proj_qk_blocks(w_cur, nxt_qk)
                if h + 1 < hpc:
                    w_next = load_w(h + 1)
                ab = attn_blocks(h - 1, cur_qk, aot_sb)
                if mut == "noattn":
                    ab = []
                elif mut == "noproj":
                    pb = []
                if mut == "noil":
                    for _, blk in ab:
                        blk()
                    for blk in pb:
                        blk()
                else:
                    interleave(ab, pb)
                cur_qk = nxt_qk

            # final slot: attn(7), with outproj(jt-1) interleaved into the
            # attn jt group so the jt_end -> outproj dependency wait is hidden
            if trunc in (None, "A"):
                ab = attn_blocks(hpc - 1, cur_qk, aot_sb)
                groups = []
                abi = 0
                for jt in range(ntq):
                    n = 2 * (jt + 1) + 1
                    groups.append(ab[abi : abi + n])
                    abi += n
                assert abi == len(ab)
                for _, blk in groups[0]:
                    blk()
                for jt in range(1, ntq):
                    interleave(
                        groups[jt],
                        outproj_blocks(jt - 1, aot_sb, wop, cdp)
                        if trunc is None else [],
                    )
                if trunc is None:
                    for blk in outproj_blocks(ntq - 1, aot_sb, wop, cdp):
                        blk()

            cdp_cm.__exit__(None, None, None)
            wop_cm.__exit__(None, None, None)
            aot_cm.__exit__(None, None, None)

    nc.compile()
    return nc


def _rope_tables(t=T):
    """cos/sin in transposed+permuted layout [128, t] (fp16).

    Partition p < 64 holds dh dim 2p (even), p >= 64 holds dh dim 2(p-64)+1;
    pair (2i, 2i+1) shares inv_freq[i], so row p uses inv_freq[p % 64].
    """
    inv_freq = 1.0 / (THETA ** (np.arange(0, DH, 2, dtype=np.float64) / DH))  # [64]
    pos = np.arange(t, dtype=np.float64)
    freqs = pos[None, :] * inv_freq[np.arange(P) % 64][:, None]  # [128, t]
    sin = np.sin(freqs)
    sin[:64] = -sin[:64]  # rotate-half sign baked into the table's top half
    return (
        np.cos(freqs).astype(np.float16),
        sin.astype(np.float16),
    )


def _perm():
    """Within-head dh permutation: even dims first, then odd dims."""
    return np.concatenate([np.arange(0, DH, 2), np.arange(1, DH, 2)])


def _tri():
    """tri[dk, dq] = 1 if dk <= dq else 0 (within-chunk causal triangle)."""
    dk = np.arange(P)[:, None]
    dq = np.arange(P)[None, :]
    return (dk <= dq).astype(np.float16)


def prep_core_inputs(x_b, Wq_g, Wk_g, Wv_g, Wo_g, t=T, hpc=HPC, npdt=np.float16):
    """Host-side input prep for one core.

    x_b: [t, D] (this core's batch); W*_g: this core's head-group slices
    (Wq/Wk/Wv: [D, hpc*DH] columns, Wo: [hpc*DH, D] rows).
    """
    d = x_b.shape[1]
    perm = _perm()
    scale = 1.0 / np.sqrt(DH)
    vg = 2
    vgw = (hpc // vg) * DH

    wq = np.empty((hpc, d, DH), npdt)
    wk = np.empty((hpc, d, DH), npdt)
    for h in range(hpc):
        blk_q = Wq_g[:, h * DH : (h + 1) * DH]
        blk_k = Wk_g[:, h * DH : (h + 1) * DH]
        wq[h] = (blk_q[:, perm] * scale).astype(npdt)
        wk[h] = blk_k[:, perm].astype(npdt)

    cos, sin = _rope_tables(t)
    return {
        "xt": np.ascontiguousarray(x_b.T).astype(npdt),
        "wq": wq,
        "wk": wk,
        "wv": np.ascontiguousarray(
            Wv_g.astype(npdt).reshape(d, vg, vgw).transpose(1, 0, 2)
        ),
        "wo": Wo_g.astype(npdt),
        "cos": cos,
        "sin": sin,
        "tri": _tri(),
        "ones": np.ones((P, P), npdt),
    }


def make_in_maps(inputs, npdt=np.float16):
    x, Wq, Wk, Wv, Wo = (
        np.asarray(inputs["x"]),
        np.asarray(inputs["Wq"]),
        np.asarray(inputs["Wk"]),
        np.asarray(inputs["Wv"]),
        np.asarray(inputs["Wo"]),
    )
    in_maps = []
    for c in range(NCORES):
        b, g = c // 2, c % 2
        cols = slice(g * HPC * DH, (g + 1) * HPC * DH)
        in_maps.append(
            prep_core_inputs(
                x[b], Wq[:, cols], Wk[:, cols], Wv[:, cols], Wo[cols, :], npdt=npdt
            )
        )
    return in_maps


def _build_sharded(nc, n_cores=NCORES):
    """Build a reusable jitted 8-core executable (bass2jax multi-core path,
    without output donation so it can be re-invoked for timing)."""
    import jax
    from jax.experimental.shard_map import shard_map
    from jax.sharding import Mesh, NamedSharding, PartitionSpec

    from concourse import bass2jax

    bass2jax.install_neuronx_cc_hook()
    partition_name = nc.partition_id_tensor.name if nc.partition_id_tensor else None
    in_names, out_names, out_avals, zero_outs = [], [], [], []
    for alloc in nc.m.functions[0].allocations:
        if not isinstance(alloc, mybir.MemoryLocationSet):
            continue
        name = alloc.memorylocations[0].name
        if alloc.kind == "ExternalInput":
            if name != partition_name:
                in_names.append(name)
        elif alloc.kind == "ExternalOutput":
            out_names.append(name)
            shape = tuple(alloc.tensor_shape)
            dtype = mybir.dt.np(alloc.dtype)
            out_avals.append(jax.core.ShapedArray(shape, dtype))
            zero_outs.append(np.zeros(shape, dtype))
    n_params = len(in_names)
    all_names = in_names + out_names
    if partition_name is not None:
        all_names = all_names + [partition_name]

    def _body(*args):
        operands = list(args)
        if partition_name is not None:
            operands.append(bass2jax.partition_id_tensor())
        outs = bass2jax._bass_exec_p.bind(
            *operands,
            out_avals=tuple(out_avals),
            in_names=tuple(all_names),
            out_names=tuple(out_names),
            lowering_input_output_aliases=(),
            sim_require_finite=True,
            sim_require_nnan=True,
            nc=nc,
        )
        return tuple(outs)

    def _chain(n):
        def f(*args):
            outs = _body(*args)
            for _ in range(n - 1):
                # 0-valued data dependency on the previous execution's first
                # output forces sequential NEFF executions on-device
                dep = (outs[0].ravel()[0] * 0).astype(args[0].dtype)
                outs = _body(args[0] + dep, *args[1:])
            return outs

        return f

    devices = jax.devices()[:n_cores]
    mesh = Mesh(np.asarray(devices), ("core",))
    in_specs = (PartitionSpec("core"),) * (n_params + len(out_names))
    out_specs = (PartitionSpec("core"),) * len(out_names)

    def _jit(body):
        return jax.jit(
            shard_map(
                body, mesh=mesh, in_specs=in_specs, out_specs=out_specs, check_rep=False
            ),
            keep_unused=True,
        )

    fn = _jit(_body)
    sharding = NamedSharding(mesh, PartitionSpec("core"))
    return fn, _jit, _chain, sharding, in_names, out_names, out_avals, zero_outs


def run_timed(nc, in_maps, reps=6, chain=0, n_cores=NCORES):
    """Run on all cores; return (per-core results, per-exec device ns)."""
    import time

    import jax

    fn, _jit, _chain, sharding, in_names, out_names, out_avals, zero_outs = (
        _build_sharded(nc, n_cores)
    )
    concat_in = [
        np.concatenate([np.asarray(in_maps[c][n]) for c in range(n_cores)], axis=0)
        for n in in_names
    ]
    concat_zeros = [
        np.zeros((n_cores * z.shape[0], *z.shape[1:]), z.dtype) for z in zero_outs
    ]
    dev_in = [jax.device_put(a, sharding) for a in concat_in]
    dev_zeros = [jax.device_put(a, sharding) for a in concat_zeros]
    out = jax.block_until_ready(fn(*dev_in, *dev_zeros))

    def _time(f):
        ts = []
        for _ in range(reps):
            t0 = time.perf_counter()
            jax.block_until_ready(f(*dev_in, *dev_zeros))
            ts.append(time.perf_counter() - t0)
        print("rep times (ms):", [f"{x * 1e3:.2f}" for x in ts])
        return min(ts)

    exec_ns = None
    if chain and chain > 1:
        fnc = _jit(_chain(chain))
        jax.block_until_ready(fnc(*dev_in, *dev_zeros))  # compile
        t1 = _time(fn)
        tn = _time(fnc)
        exec_ns = int((tn - t1) / (chain - 1) * 1e9)
        print(f"single call: {t1 * 1e3:.2f} ms, chain-{chain}: {tn * 1e3:.2f} ms")
    else:
        exec_ns = int(_time(fn) * 1e9)
    results = [
        {
            name: np.asarray(out[i]).reshape(n_cores, *out_avals[i].shape)[c]
            for i, name in enumerate(out_names)
        }
        for c in range(n_cores)
    ]
    return results, exec_ns


def kernel(x, Wq, Wk, Wv, Wo):
    nc = build()
    in_maps = make_in_maps({"x": x, "Wq": Wq, "Wk": Wk, "Wv": Wv, "Wo": Wo})
    results = run_bass_kernel_spmd(nc, in_maps, core_ids=list(range(NCORES))).results
    out = np.empty((B, T, D), np.float32)
    for b in range(B):
        out[b] = (
            results[2 * b]["y"].astype(np.float32)
            + results[2 * b + 1]["y"].astype(np.float32)
        ).T
    return out
